# revision 30
# baseline (speedup 1.0000x reference)
"""Trainium2 Bass kernel for IntrinsicSignalSynthesizer.

Data-parallel over 8 NeuronCores: batch 16384 -> 8 x 2048 rows.

Design notes (v5):
- MLP matmuls are feature-major ([128 feat, kchunk, rows] tiles): every
  matmul contracts over the partition dim at the bf16 stream roofline.
- Per-row reductions (sum e^p, sum p*e^p, sum a^2) run OFF the PE: the
  inputs are DMA'd a second time in row-major layout and reduced along
  the free dim with DVE tensor_scalar/scalar_tensor_tensor accum_out.
- Per-row scalar finishing is row-major [128 rows, 4 subblocks] per
  row-tile; the PE-produced [1, 512] scalars (3 MLP heads + compression
  sum) are staged at quadrant partitions {0,32,64,96} of one tile and
  moved row-major with a single [128,128] PE transpose per subblock.
- DMA order: pattern memory + tile-0 feature-major inputs first, then
  small weights, then the 4 MiB dissonance weight in 4 chunks - the PE
  starts on sims/small MLPs ~4 us in while the big weight streams.
- Finishing is per-row-tile so the epilogue of the last tile is short.
"""
import sys
sys.path.insert(0, '/opt/trn_rl_repo')

import numpy as np
import ml_dtypes

import concourse.bass as bass
import concourse.mybir as mybir
import concourse.tile as tile
from concourse.bass_utils import run_bass_kernel_spmd

BF16 = mybir.dt.bfloat16
F32 = mybir.dt.float32
AF = mybir.ActivationFunctionType
ALU = mybir.AluOpType
AX = mybir.AxisListType

B, D = 16384, 1024
MEM = 100
NCORES = 8
ROWS = B // NCORES            # 2048 rows per core
NT = 512                      # rows per row-tile
NTILES = ROWS // NT           # 4
NSUB = ROWS // 128            # 16 row-subblocks per core
KD = D // 128                 # 8 feature chunks

MAX_WAITS = 1


def _split_excess_waits(nc):
    # walrus CTRL encoding caps sync waits per instruction; the TileContext
    # tail drain can exceed that. Move excess waits onto preceding NoOps.
    for fn in nc.m.functions:
        for bb in fn.blocks:
            if not isinstance(bb, mybir.BasicBlock):
                continue
            insts = bb.instructions
            i = 0
            while i < len(insts):
                ins = insts[i]
                si = getattr(ins, 'sync_info', None)
                waits = list(si.on_wait) if si is not None and si.on_wait else []
                if len(waits) > MAX_WAITS:
                    chunks = [waits[j:j + MAX_WAITS]
                              for j in range(0, len(waits), MAX_WAITS)]
                    si.on_wait = chunks[-1]
                    new_ops = [
                        mybir.InstNoOp(
                            name=f"{ins.name}-waitsplit-{k}",
                            engine=ins.engine,
                            sync_info=mybir.SyncInfo(on_wait=ch, on_update=[]),
                            bass_nofuse=True,
                        )
                        for k, ch in enumerate(chunks[:-1])
                    ]
                    insts[i:i] = new_ops
                    i += len(new_ops)
                i += 1


def build_kernel(reps: int = 1):
    assert reps == 1, "tile-0 input prefetch assumes a single pass"
    nc = bass.Bass()

    pt_d = nc.dram_tensor("pt", [D, ROWS], BF16, kind="ExternalInput")
    at_d = nc.dram_tensor("at", [D, ROWS], BF16, kind="ExternalInput")
    prm_d = nc.dram_tensor("prm", [128, NSUB, D], BF16, kind="ExternalInput")
    arm_d = nc.dram_tensor("arm", [128, NSUB, D], BF16, kind="ExternalInput")
    wd_d = nc.dram_tensor("wd", [2 * D, D], BF16, kind="ExternalInput")
    wu_d = nc.dram_tensor("wu", [D, D // 2], BF16, kind="ExternalInput")
    wn_d = nc.dram_tensor("wn", [D, D // 2], BF16, kind="ExternalInput")
    wc1_d = nc.dram_tensor("wc1", [D, D // 4], BF16, kind="ExternalInput")
    wc2_d = nc.dram_tensor("wc2", [D // 4, D], BF16, kind="ExternalInput")
    wd2_d = nc.dram_tensor("wd2", [D, 1], BF16, kind="ExternalInput")
    wu2_d = nc.dram_tensor("wu2", [D // 2, 1], BF16, kind="ExternalInput")
    wn2_d = nc.dram_tensor("wn2", [D // 2, 1], BF16, kind="ExternalInput")
    mh_d = nc.dram_tensor("mh", [D, MEM], BF16, kind="ExternalInput")
    ones_d = nc.dram_tensor("ones", [128, 1], BF16, kind="ExternalInput")
    ident_d = nc.dram_tensor("ident", [128, 128], F32, kind="ExternalInput")
    bd1_d = nc.dram_tensor("bd1", [128, KD], F32, kind="ExternalInput")
    bu1_d = nc.dram_tensor("bu1", [128, 4], F32, kind="ExternalInput")
    bn1_d = nc.dram_tensor("bn1", [128, 4], F32, kind="ExternalInput")
    bc1_d = nc.dram_tensor("bc1", [128, 2], F32, kind="ExternalInput")
    bc2_d = nc.dram_tensor("bc2", [128, KD], F32, kind="ExternalInput")
    bh_d = nc.dram_tensor("bh", [128, 3], F32, kind="ExternalInput")  # d2,u2,n2
    out_d = nc.dram_tensor("out", [4, ROWS], F32, kind="ExternalOutput")

    with tile.TileContext(nc) as tc:
        import contextlib
        ctx = contextlib.ExitStack()
        with ctx:
            W = ctx.enter_context(tc.tile_pool(name="weights", bufs=1))
            io = ctx.enter_context(tc.tile_pool(name="io", bufs=3))
            io2 = ctx.enter_context(tc.tile_pool(name="io2", bufs=2))
            big = ctx.enter_context(tc.tile_pool(name="big", bufs=2))
            sm = ctx.enter_context(tc.tile_pool(name="sm", bufs=2))
            mm = ctx.enter_context(tc.tile_pool(name="mmp", bufs=3, space="PSUM"))
            vec = ctx.enter_context(tc.tile_pool(name="vecp", bufs=2, space="PSUM"))
            simp = ctx.enter_context(tc.tile_pool(name="simp", bufs=1, space="PSUM"))
            trp = ctx.enter_context(tc.tile_pool(name="trp", bufs=1, space="PSUM"))

            # --- DMA emission order == start order. First what the PE needs
            # first: pattern memory, tile-0 FM inputs, small weights; the big
            # dissonance weight last, streaming under early compute.
            mh = W.tile([128, KD, MEM], BF16)
            nc.sync.dma_start(mh, mh_d.rearrange("(k p) m -> p k m", p=128))

            def load_fm(t):
                rs = slice(t * NT, (t + 1) * NT)
                at = io.tile([128, KD, NT], BF16, tag="at")
                nc.sync.dma_start(
                    at, at_d[:, rs].rearrange("(k p) r -> p k r", p=128))
                pt = io.tile([128, KD, NT], BF16, tag="pt")
                nc.sync.dma_start(
                    pt, pt_d[:, rs].rearrange("(k p) r -> p k r", p=128))
                return at, pt

            def load_rm(t):
                prm = io2.tile([128, 4, D], BF16, tag="prm")
                nc.sync.dma_start(prm, prm_d[:, 4 * t:4 * t + 4, :])
                arm = io2.tile([128, 4, D], BF16, tag="arm")
                nc.sync.dma_start(arm, arm_d[:, 4 * t:4 * t + 4, :])
                return prm, arm

            fm0 = load_fm(0)

            wu = W.tile([128, KD, D // 2], BF16)
            nc.sync.dma_start(wu, wu_d.rearrange("(k p) m -> p k m", p=128))
            # dissonance weight next (4 chunk-group DMAs): it gates the bulk
            # of tile-0 PE work
            wd = []
            for g in range(4):
                wg = W.tile([128, 4, D], BF16, tag=f"wdc{g}")
                nc.sync.dma_start(
                    wg, wd_d[g * 512:(g + 1) * 512, :].rearrange(
                        "(k p) m -> p k m", p=128))
                wd.append(wg)
            wn = W.tile([128, KD, D // 2], BF16)
            nc.sync.dma_start(wn, wn_d.rearrange("(k p) m -> p k m", p=128))
            wc1 = W.tile([128, KD, D // 4], BF16)
            nc.sync.dma_start(wc1, wc1_d.rearrange("(k p) m -> p k m", p=128))
            wc2 = W.tile([128, 2, D], BF16)
            nc.sync.dma_start(wc2, wc2_d.rearrange("(k p) m -> p k m", p=128))
            wd2 = W.tile([128, KD, 1], BF16)
            nc.sync.dma_start(wd2, wd2_d.rearrange("(k p) m -> p k m", p=128))
            wu2 = W.tile([128, 4, 1], BF16)
            nc.sync.dma_start(wu2, wu2_d.rearrange("(k p) m -> p k m", p=128))
            wn2 = W.tile([128, 4, 1], BF16)
            nc.sync.dma_start(wn2, wn2_d.rearrange("(k p) m -> p k m", p=128))
            ones = W.tile([128, 1], BF16)
            nc.sync.dma_start(ones, ones_d[:])
            ident = W.tile([128, 128], F32)
            nc.sync.dma_start(ident, ident_d[:])
            bd1 = W.tile([128, KD], F32)
            nc.sync.dma_start(bd1, bd1_d[:])
            bu1 = W.tile([128, 4], F32)
            nc.sync.dma_start(bu1, bu1_d[:])
            bn1 = W.tile([128, 4], F32)
            nc.sync.dma_start(bn1, bn1_d[:])
            bc1 = W.tile([128, 2], F32)
            nc.sync.dma_start(bc1, bc1_d[:])
            bc2 = W.tile([128, KD], F32)
            nc.sync.dma_start(bc2, bc2_d[:])
            bh = W.tile([128, 3], F32)
            nc.sync.dma_start(bh, bh_d[:])
            rm0 = load_rm(0)

            def wdk(k):
                return wd[k // 4][:, k % 4, :]

            for t in range(NTILES):
                at, pt = fm0 if t == 0 else load_fm(t)
                prm, arm = rm0 if t == 0 else load_rm(t)
                raw = sm.tile([128, 4], F32, tag="raw")
                zZ = sm.tile([128, 4], F32, tag="zZ")
                zS = sm.tile([128, 4], F32, tag="zS")
                na2 = sm.tile([128, 4], F32, tag="na2")

                # --- sims (needs only mh + at): row-major max cos sim
                for s in range(4):
                    pss = simp.tile([128, MEM], F32, tag="simp")
                    for k in range(KD):
                        nc.tensor.matmul(
                            pss, at[:, k, s * 128:(s + 1) * 128],
                            mh[:, k, :],
                            start=(k == 0), stop=(k == KD - 1))
                    nc.vector.reduce_max(raw[:, s:s + 1], pss, axis=AX.X)

                # per-row scalars staged at quadrant-aligned partitions
                # {0,32,64,96} (engine writes must be 32-aligned)
                z4 = sm.tile([128, NT], F32, tag="z4")

                # --- uncertainty MLP
                hu = big.tile([128, 4, NT], BF16, tag="hu")
                for j in range(4):
                    ps = mm.tile([128, NT], F32, tag="mm")
                    for k in range(KD):
                        nc.tensor.matmul(ps, wu[:, k, j * 128:(j + 1) * 128],
                                         pt[:, k, :],
                                         start=(k == 0), stop=(k == KD - 1))
                    nc.scalar.activation(hu[:, j, :], ps, AF.Relu,
                                         bias=bu1[:, j:j + 1])
                zu = vec.tile([1, NT], F32, tag="vec")
                for j in range(4):
                    nc.tensor.matmul(zu, wu2[:, j, :], hu[:, j, :],
                                     start=(j == 0), stop=(j == 3))
                nc.scalar.activation(z4[32:33, :], zu, AF.Identity,
                                     bias=bh[0:1, 1:2])

                # --- entropy sums + ||a||^2: row-major free-dim reduces
                # fused into DVE elementwise ops via accum_out
                for s in range(4):
                    e = sm.tile([128, D], BF16, tag="e")
                    nc.scalar.activation(e, prm[:, s, :], AF.Exp)
                    e2 = sm.tile([128, D], BF16, tag="e2")
                    nc.vector.tensor_scalar(
                        e2, e, 1.0, 0.0, op0=ALU.mult, op1=ALU.add,
                        accum_out=zZ[:, s:s + 1])
                    nc.vector.scalar_tensor_tensor(
                        e2, e, 1.0, prm[:, s, :],
                        op0=ALU.mult, op1=ALU.mult,
                        accum_out=zS[:, s:s + 1])
                    nc.vector.scalar_tensor_tensor(
                        e2, arm[:, s, :], 1.0, arm[:, s, :],
                        op0=ALU.mult, op1=ALU.mult,
                        accum_out=na2[:, s:s + 1])

                # --- novelty MLP
                hn = big.tile([128, 4, NT], BF16, tag="hn")
                for j in range(4):
                    ps = mm.tile([128, NT], F32, tag="mm")
                    for k in range(KD):
                        nc.tensor.matmul(ps, wn[:, k, j * 128:(j + 1) * 128],
                                         at[:, k, :],
                                         start=(k == 0), stop=(k == KD - 1))
                    nc.scalar.activation(hn[:, j, :], ps, AF.Relu,
                                         bias=bn1[:, j:j + 1])
                zn = vec.tile([1, NT], F32, tag="vec")
                for j in range(4):
                    nc.tensor.matmul(zn, wn2[:, j, :], hn[:, j, :],
                                     start=(j == 0), stop=(j == 3))
                nc.scalar.activation(z4[64:65, :], zn, AF.Identity,
                                     bias=bh[0:1, 2:3])

                # --- compression
                hc = big.tile([128, 2, NT], BF16, tag="hc")
                for j in range(2):
                    ps = mm.tile([128, NT], F32, tag="mm")
                    for k in range(KD):
                        nc.tensor.matmul(ps, wc1[:, k, j * 128:(j + 1) * 128],
                                         pt[:, k, :],
                                         start=(k == 0), stop=(k == KD - 1))
                    nc.scalar.activation(hc[:, j, :], ps, AF.Relu,
                                         bias=bc1[:, j:j + 1])
                ms = vec.tile([1, NT], F32, tag="vec")
                for j in range(KD):
                    psr = mm.tile([128, NT], F32, tag="mm")
                    for k in range(2):
                        nc.tensor.matmul(psr, wc2[:, k, j * 128:(j + 1) * 128],
                                         hc[:, k, :],
                                         start=(k == 0), stop=(k == 1))
                    dj = sm.tile([128, NT], BF16, tag="dj")
                    # dj = (recon + bc2) - p  (sign-flipped diff; squared)
                    nc.vector.scalar_tensor_tensor(
                        dj, psr, bc2[:, j:j + 1], pt[:, j, :],
                        op0=ALU.add, op1=ALU.subtract)
                    dsq = sm.tile([128, NT], BF16, tag="dsq")
                    nc.scalar.activation(dsq, dj, AF.Square)
                    nc.tensor.matmul(ms, ones, dsq,
                                     start=(j == 0), stop=(j == KD - 1))
                nc.vector.tensor_copy(z4[96:97, :], ms)

                # --- dissonance
                hd = big.tile([128, KD, NT], BF16, tag="hd")
                for j in range(KD):
                    ps = mm.tile([128, NT], F32, tag="mm")
                    for k in range(16):
                        x = pt[:, k, :] if k < KD else at[:, k - KD, :]
                        nc.tensor.matmul(ps, wdk(k)[:, j * 128:(j + 1) * 128],
                                         x, start=(k == 0), stop=(k == 15))
                    nc.scalar.activation(hd[:, j, :], ps, AF.Relu,
                                         bias=bd1[:, j:j + 1])
                zd = vec.tile([1, NT], F32, tag="vec")
                for j in range(KD):
                    nc.tensor.matmul(zd, wd2[:, j, :], hd[:, j, :],
                                     start=(j == 0), stop=(j == KD - 1))
                nc.scalar.activation(z4[0:1, :], zd, AF.Identity,
                                     bias=bh[0:1, 0:1])

                # --- move the 4 per-row scalars row-major: one [128,128]
                # PE transpose per subblock; quantities land on cols
                # {0,32,64,96} (zd, zu, zn, ms).
                zAll = sm.tile([128, 4, 4], F32, tag="zAll")
                for s in range(4):
                    ztr = trp.tile([128, 4, 32], F32, tag="ztr")
                    nc.tensor.transpose(
                        ztr, z4[:, s * 128:(s + 1) * 128], ident)
                    nc.vector.tensor_copy(zAll[:, s, :], ztr[:, :, 0])

                # --- per-tile finishing, row-major [128 rows, 4 subs]
                zdR = zAll[:, :, 0]
                zuR = zAll[:, :, 1]
                znR = zAll[:, :, 2]
                msR = zAll[:, :, 3]
                fin = sm.tile([128, 12, 4], F32, tag="fin")
                tmp1, tmp2, tmp3 = fin[:, 3, :], fin[:, 4, :], fin[:, 5, :]
                uncR, novR = fin[:, 6, :], fin[:, 7, :]
                tmp4, tmp5 = fin[:, 8, :], fin[:, 9, :]

                # softplus(z + b) = Ln(1 + Exp(z + b)); head biases were
                # already folded in by the z4 staging copies. One batched
                # Exp + Ln over all three heads [128, 4 subs, 3].
                sp3 = sm.tile([128, 4, 3], F32, tag="sp3")
                ex3 = sm.tile([128, 4, 3], F32, tag="ex3")
                nc.scalar.activation(ex3, zAll[:, :, 0:3], AF.Exp)
                nc.scalar.activation(sp3, ex3, AF.Ln, bias=1.0)
                spD, spU, spN = sp3[:, :, 0], sp3[:, :, 1], sp3[:, :, 2]

                # uncertainty = spU + 0.1 * (lnZ - zS/Z)
                nc.scalar.activation(tmp1, zZ, AF.Ln)      # lnZ
                nc.vector.reciprocal(tmp2, zZ)             # 1/Z
                nc.vector.tensor_mul(tmp3, zS, tmp2)       # zS/Z
                nc.vector.tensor_tensor(tmp1, tmp1, tmp3, ALU.subtract)
                nc.vector.scalar_tensor_tensor(
                    uncR, tmp1, 0.1, spU, op0=ALU.mult, op1=ALU.add)

                # novelty = 0.7*(1 - raw/||a||) + 0.3*spN
                nc.scalar.activation(tmp4, na2, AF.Ln)
                nc.scalar.activation(tmp5, tmp4, AF.Exp, scale=-0.5)  # 1/||a||
                nc.vector.tensor_mul(tmp4, raw, tmp5)      # cos
                nc.vector.tensor_scalar_mul(tmp5, spN, 0.3)
                nc.vector.scalar_tensor_tensor(
                    novR, tmp4, -0.7, tmp5, op0=ALU.mult, op1=ALU.add)
                nc.vector.tensor_scalar_add(novR, novR, 0.7)

                nc.vector.tensor_scalar_mul(tmp5, msR, 1.0 / D)     # comp

                # --- assemble this tile's [4, NT] output slice
                rs = slice(t * NT, (t + 1) * NT)
                for q, src in enumerate((spD, uncR, novR, tmp5)):
                    oT = trp.tile([4, 128], F32, tag="oT")
                    nc.tensor.transpose(oT, src, ident)
                    ob = sm.tile([4, 128], F32, tag="ob")
                    nc.scalar.copy(ob, oT)
                    nc.sync.dma_start(
                        out_d[q:q + 1, rs].rearrange(
                            "a (s r) -> (a s) r", s=4),
                        ob)

    _split_excess_waits(nc)
    return nc


def _prep_inputs(prediction, actual, pattern_memory,
                 W_d1, b_d1, W_d2, b_d2, W_u1, b_u1, W_u2, b_u2,
                 W_n1, b_n1, W_n2, b_n2, W_c1, b_c1, W_c2, b_c2):
    bf = ml_dtypes.bfloat16

    def t_bf(a):  # transposed contiguous bf16
        return np.ascontiguousarray(np.asarray(a, np.float32).T).astype(bf)

    mnorm = np.maximum(np.linalg.norm(
        np.asarray(pattern_memory, np.float32), axis=1), 1e-8)
    mhat = np.asarray(pattern_memory, np.float32) / mnorm[:, None]

    def fold_bias(b, chunks):
        return np.ascontiguousarray(
            np.asarray(b, np.float32).reshape(chunks, 128).T)

    bh = np.empty((128, 3), np.float32)
    bh[:, 0] = float(np.asarray(b_d2).reshape(-1)[0])
    bh[:, 1] = float(np.asarray(b_u2).reshape(-1)[0])
    bh[:, 2] = float(np.asarray(b_n2).reshape(-1)[0])

    shared = {
        "wd": t_bf(W_d1), "wu": t_bf(W_u1), "wn": t_bf(W_n1),
        "wc1": t_bf(W_c1), "wc2": t_bf(W_c2),
        "wd2": t_bf(W_d2), "wu2": t_bf(W_u2), "wn2": t_bf(W_n2),
        "mh": t_bf(mhat),
        "ones": np.ones((128, 1), bf),
        "ident": np.eye(128, dtype=np.float32),
        "bd1": fold_bias(b_d1, KD), "bu1": fold_bias(b_u1, 4),
        "bn1": fold_bias(b_n1, 4), "bc1": fold_bias(b_c1, 2),
        "bc2": fold_bias(b_c2, KD),
        "bh": bh,
    }
    p32 = np.asarray(prediction, np.float32)
    a32 = np.asarray(actual, np.float32)
    in_maps = []
    for c in range(NCORES):
        rows = slice(c * ROWS, (c + 1) * ROWS)
        m = dict(shared)
        m["pt"] = np.ascontiguousarray(p32[rows].T).astype(bf)
        m["at"] = np.ascontiguousarray(a32[rows].T).astype(bf)
        m["prm"] = np.ascontiguousarray(
            p32[rows].reshape(NSUB, 128, D).transpose(1, 0, 2)).astype(bf)
        m["arm"] = np.ascontiguousarray(
            a32[rows].reshape(NSUB, 128, D).transpose(1, 0, 2)).astype(bf)
        in_maps.append(m)
    return in_maps


_NC_CACHE = {}


def kernel(**inputs) -> np.ndarray:
    in_maps = _prep_inputs(**inputs)
    if 'nc' not in _NC_CACHE:
        _NC_CACHE['nc'] = build_kernel(reps=1)
    nc = _NC_CACHE['nc']
    res = run_bass_kernel_spmd(nc, in_maps, core_ids=list(range(NCORES)))
    out = np.empty((B, 4), np.float32)
    for c in range(NCORES):
        out[c * ROWS:(c + 1) * ROWS, :] = res.results[c]["out"].T
    return out


# revision 31
# speedup vs baseline: 1.0431x; 1.0431x over previous
"""Trainium2 Bass kernel for IntrinsicSignalSynthesizer.

Data-parallel over 8 NeuronCores: batch 16384 -> 8 x 2048 rows.

Design notes (v5):
- MLP matmuls are feature-major ([128 feat, kchunk, rows] tiles): every
  matmul contracts over the partition dim at the bf16 stream roofline.
- Per-row reductions (sum e^p, sum p*e^p, sum a^2) run OFF the PE: the
  inputs are DMA'd a second time in row-major layout and reduced along
  the free dim with DVE tensor_scalar/scalar_tensor_tensor accum_out.
- Per-row scalar finishing is row-major [128 rows, 4 subblocks] per
  row-tile; the PE-produced [1, 512] scalars (3 MLP heads + compression
  sum) are staged at quadrant partitions {0,32,64,96} of one tile and
  moved row-major with a single [128,128] PE transpose per subblock.
- DMA order: pattern memory + tile-0 feature-major inputs first, then
  small weights, then the 4 MiB dissonance weight in 4 chunks - the PE
  starts on sims/small MLPs ~4 us in while the big weight streams.
- Finishing is per-row-tile so the epilogue of the last tile is short.
"""
import sys
sys.path.insert(0, '/opt/trn_rl_repo')

import numpy as np
import ml_dtypes

import concourse.bass as bass
import concourse.mybir as mybir
import concourse.tile as tile
from concourse.bass_utils import run_bass_kernel_spmd

BF16 = mybir.dt.bfloat16
F32 = mybir.dt.float32
AF = mybir.ActivationFunctionType
ALU = mybir.AluOpType
AX = mybir.AxisListType

B, D = 16384, 1024
MEM = 100
NCORES = 8
ROWS = B // NCORES            # 2048 rows per core
NT = 512                      # rows per row-tile
NTILES = ROWS // NT           # 4
NSUB = ROWS // 128            # 16 row-subblocks per core
KD = D // 128                 # 8 feature chunks

MAX_WAITS = 1


def _split_excess_waits(nc):
    # walrus CTRL encoding caps sync waits per instruction; the TileContext
    # tail drain can exceed that. Move excess waits onto preceding NoOps.
    for fn in nc.m.functions:
        for bb in fn.blocks:
            if not isinstance(bb, mybir.BasicBlock):
                continue
            insts = bb.instructions
            i = 0
            while i < len(insts):
                ins = insts[i]
                si = getattr(ins, 'sync_info', None)
                waits = list(si.on_wait) if si is not None and si.on_wait else []
                if len(waits) > MAX_WAITS:
                    chunks = [waits[j:j + MAX_WAITS]
                              for j in range(0, len(waits), MAX_WAITS)]
                    si.on_wait = chunks[-1]
                    new_ops = [
                        mybir.InstNoOp(
                            name=f"{ins.name}-waitsplit-{k}",
                            engine=ins.engine,
                            sync_info=mybir.SyncInfo(on_wait=ch, on_update=[]),
                            bass_nofuse=True,
                        )
                        for k, ch in enumerate(chunks[:-1])
                    ]
                    insts[i:i] = new_ops
                    i += len(new_ops)
                i += 1


def build_kernel(reps: int = 1):
    assert reps == 1, "tile-0 input prefetch assumes a single pass"
    nc = bass.Bass()

    pt_d = nc.dram_tensor("pt", [D, ROWS], BF16, kind="ExternalInput")
    at_d = nc.dram_tensor("at", [D, ROWS], BF16, kind="ExternalInput")
    prm_d = nc.dram_tensor("prm", [128, NSUB, D], BF16, kind="ExternalInput")
    arm_d = nc.dram_tensor("arm", [128, NSUB, D], BF16, kind="ExternalInput")
    wd_d = nc.dram_tensor("wd", [2 * D, D], BF16, kind="ExternalInput")
    wu_d = nc.dram_tensor("wu", [D, D // 2], BF16, kind="ExternalInput")
    wn_d = nc.dram_tensor("wn", [D, D // 2], BF16, kind="ExternalInput")
    wc1_d = nc.dram_tensor("wc1", [D, D // 4], BF16, kind="ExternalInput")
    wc2_d = nc.dram_tensor("wc2", [D // 4, D], BF16, kind="ExternalInput")
    wd2_d = nc.dram_tensor("wd2", [D, 1], BF16, kind="ExternalInput")
    wu2_d = nc.dram_tensor("wu2", [D // 2, 1], BF16, kind="ExternalInput")
    wn2_d = nc.dram_tensor("wn2", [D // 2, 1], BF16, kind="ExternalInput")
    mh_d = nc.dram_tensor("mh", [D, MEM], BF16, kind="ExternalInput")
    ones_d = nc.dram_tensor("ones", [128, 1], BF16, kind="ExternalInput")
    ident_d = nc.dram_tensor("ident", [128, 128], F32, kind="ExternalInput")
    bd1_d = nc.dram_tensor("bd1", [128, KD], F32, kind="ExternalInput")
    bu1_d = nc.dram_tensor("bu1", [128, 4], F32, kind="ExternalInput")
    bn1_d = nc.dram_tensor("bn1", [128, 4], F32, kind="ExternalInput")
    bc1_d = nc.dram_tensor("bc1", [128, 2], F32, kind="ExternalInput")
    bc2_d = nc.dram_tensor("bc2", [128, KD], F32, kind="ExternalInput")
    bh_d = nc.dram_tensor("bh", [128, 3], F32, kind="ExternalInput")  # d2,u2,n2
    out_d = nc.dram_tensor("out", [4, ROWS], F32, kind="ExternalOutput")

    with tile.TileContext(nc) as tc:
        import contextlib
        ctx = contextlib.ExitStack()
        with ctx:
            W = ctx.enter_context(tc.tile_pool(name="weights", bufs=1))
            io = ctx.enter_context(tc.tile_pool(name="io", bufs=3))
            io2 = ctx.enter_context(tc.tile_pool(name="io2", bufs=2))
            big = ctx.enter_context(tc.tile_pool(name="big", bufs=2))
            sm = ctx.enter_context(tc.tile_pool(name="sm", bufs=2))
            mm = ctx.enter_context(tc.tile_pool(name="mmp", bufs=3, space="PSUM"))
            vec = ctx.enter_context(tc.tile_pool(name="vecp", bufs=2, space="PSUM"))
            simp = ctx.enter_context(tc.tile_pool(name="simp", bufs=1, space="PSUM"))
            trp = ctx.enter_context(tc.tile_pool(name="trp", bufs=1, space="PSUM"))

            # --- DMA emission order == start order. First what the PE needs
            # first: pattern memory, tile-0 FM inputs, small weights; the big
            # dissonance weight last, streaming under early compute.
            mh = W.tile([128, KD, MEM], BF16)
            nc.sync.dma_start(mh, mh_d.rearrange("(k p) m -> p k m", p=128))

            def load_fm(t):
                rs = slice(t * NT, (t + 1) * NT)
                at = io.tile([128, KD, NT], BF16, tag="at")
                nc.sync.dma_start(
                    at, at_d[:, rs].rearrange("(k p) r -> p k r", p=128))
                pt = io.tile([128, KD, NT], BF16, tag="pt")
                nc.sync.dma_start(
                    pt, pt_d[:, rs].rearrange("(k p) r -> p k r", p=128))
                return at, pt

            def load_rm(t):
                prm = io2.tile([128, 4, D], BF16, tag="prm")
                nc.sync.dma_start(prm, prm_d[:, 4 * t:4 * t + 4, :])
                arm = io2.tile([128, 4, D], BF16, tag="arm")
                nc.sync.dma_start(arm, arm_d[:, 4 * t:4 * t + 4, :])
                return prm, arm

            fm0 = load_fm(0)

            wu = W.tile([128, KD, D // 2], BF16)
            nc.sync.dma_start(wu, wu_d.rearrange("(k p) m -> p k m", p=128))
            wn = W.tile([128, KD, D // 2], BF16)
            nc.sync.dma_start(wn, wn_d.rearrange("(k p) m -> p k m", p=128))
            wc1 = W.tile([128, KD, D // 4], BF16)
            nc.sync.dma_start(wc1, wc1_d.rearrange("(k p) m -> p k m", p=128))
            wc2 = W.tile([128, 2, D], BF16)
            nc.sync.dma_start(wc2, wc2_d.rearrange("(k p) m -> p k m", p=128))
            wd2 = W.tile([128, KD, 1], BF16)
            nc.sync.dma_start(wd2, wd2_d.rearrange("(k p) m -> p k m", p=128))
            wu2 = W.tile([128, 4, 1], BF16)
            nc.sync.dma_start(wu2, wu2_d.rearrange("(k p) m -> p k m", p=128))
            wn2 = W.tile([128, 4, 1], BF16)
            nc.sync.dma_start(wn2, wn2_d.rearrange("(k p) m -> p k m", p=128))
            ones = W.tile([128, 1], BF16)
            nc.sync.dma_start(ones, ones_d[:])
            ident = W.tile([128, 128], F32)
            nc.sync.dma_start(ident, ident_d[:])
            bd1 = W.tile([128, KD], F32)
            nc.sync.dma_start(bd1, bd1_d[:])
            bu1 = W.tile([128, 4], F32)
            nc.sync.dma_start(bu1, bu1_d[:])
            bn1 = W.tile([128, 4], F32)
            nc.sync.dma_start(bn1, bn1_d[:])
            bc1 = W.tile([128, 2], F32)
            nc.sync.dma_start(bc1, bc1_d[:])
            bc2 = W.tile([128, KD], F32)
            nc.sync.dma_start(bc2, bc2_d[:])
            bh = W.tile([128, 3], F32)
            nc.sync.dma_start(bh, bh_d[:])
            rm0 = load_rm(0)
            # dissonance weight last (4 chunk-group DMAs), streaming in under
            # the sims/small-MLP work of tile 0
            wd = []
            for g in range(4):
                wg = W.tile([128, 4, D], BF16, tag=f"wdc{g}")
                nc.sync.dma_start(
                    wg, wd_d[g * 512:(g + 1) * 512, :].rearrange(
                        "(k p) m -> p k m", p=128))
                wd.append(wg)

            def wdk(k):
                return wd[k // 4][:, k % 4, :]

            for t in range(NTILES):
                at, pt = fm0 if t == 0 else load_fm(t)
                prm, arm = rm0 if t == 0 else load_rm(t)
                raw = sm.tile([128, 4], F32, tag="raw")
                zZ = sm.tile([128, 4], F32, tag="zZ")
                zS = sm.tile([128, 4], F32, tag="zS")
                na2 = sm.tile([128, 4], F32, tag="na2")

                # --- sims (needs only mh + at): row-major max cos sim
                for s in range(4):
                    pss = simp.tile([128, MEM], F32, tag="simp")
                    for k in range(KD):
                        nc.tensor.matmul(
                            pss, at[:, k, s * 128:(s + 1) * 128],
                            mh[:, k, :],
                            start=(k == 0), stop=(k == KD - 1))
                    nc.vector.reduce_max(raw[:, s:s + 1], pss, axis=AX.X)

                # per-row scalars staged at quadrant-aligned partitions
                # {0,32,64,96} (engine writes must be 32-aligned)
                z4 = sm.tile([128, NT], F32, tag="z4")

                # --- uncertainty MLP
                hu = big.tile([128, 4, NT], BF16, tag="hu")
                for j in range(4):
                    ps = mm.tile([128, NT], F32, tag="mm")
                    for k in range(KD):
                        nc.tensor.matmul(ps, wu[:, k, j * 128:(j + 1) * 128],
                                         pt[:, k, :],
                                         start=(k == 0), stop=(k == KD - 1))
                    nc.scalar.activation(hu[:, j, :], ps, AF.Relu,
                                         bias=bu1[:, j:j + 1])
                zu = vec.tile([1, NT], F32, tag="vec")
                for j in range(4):
                    nc.tensor.matmul(zu, wu2[:, j, :], hu[:, j, :],
                                     start=(j == 0), stop=(j == 3))
                nc.scalar.activation(z4[32:33, :], zu, AF.Identity,
                                     bias=bh[0:1, 1:2])

                # --- entropy sums + ||a||^2: row-major free-dim reduces
                # fused into DVE elementwise ops via accum_out
                for s in range(4):
                    e = sm.tile([128, D], BF16, tag="e")
                    nc.scalar.activation(e, prm[:, s, :], AF.Exp)
                    e2 = sm.tile([128, D], BF16, tag="e2")
                    nc.vector.tensor_scalar(
                        e2, e, 1.0, 0.0, op0=ALU.mult, op1=ALU.add,
                        accum_out=zZ[:, s:s + 1])
                    nc.vector.scalar_tensor_tensor(
                        e2, e, 1.0, prm[:, s, :],
                        op0=ALU.mult, op1=ALU.mult,
                        accum_out=zS[:, s:s + 1])
                    nc.vector.scalar_tensor_tensor(
                        e2, arm[:, s, :], 1.0, arm[:, s, :],
                        op0=ALU.mult, op1=ALU.mult,
                        accum_out=na2[:, s:s + 1])

                # --- novelty MLP
                hn = big.tile([128, 4, NT], BF16, tag="hn")
                for j in range(4):
                    ps = mm.tile([128, NT], F32, tag="mm")
                    for k in range(KD):
                        nc.tensor.matmul(ps, wn[:, k, j * 128:(j + 1) * 128],
                                         at[:, k, :],
                                         start=(k == 0), stop=(k == KD - 1))
                    nc.scalar.activation(hn[:, j, :], ps, AF.Relu,
                                         bias=bn1[:, j:j + 1])
                zn = vec.tile([1, NT], F32, tag="vec")
                for j in range(4):
                    nc.tensor.matmul(zn, wn2[:, j, :], hn[:, j, :],
                                     start=(j == 0), stop=(j == 3))
                nc.scalar.activation(z4[64:65, :], zn, AF.Identity,
                                     bias=bh[0:1, 2:3])

                # --- compression
                hc = big.tile([128, 2, NT], BF16, tag="hc")
                for j in range(2):
                    ps = mm.tile([128, NT], F32, tag="mm")
                    for k in range(KD):
                        nc.tensor.matmul(ps, wc1[:, k, j * 128:(j + 1) * 128],
                                         pt[:, k, :],
                                         start=(k == 0), stop=(k == KD - 1))
                    nc.scalar.activation(hc[:, j, :], ps, AF.Relu,
                                         bias=bc1[:, j:j + 1])
                ms = vec.tile([1, NT], F32, tag="vec")
                for j in range(KD):
                    psr = mm.tile([128, NT], F32, tag="mm")
                    for k in range(2):
                        nc.tensor.matmul(psr, wc2[:, k, j * 128:(j + 1) * 128],
                                         hc[:, k, :],
                                         start=(k == 0), stop=(k == 1))
                    dj = sm.tile([128, NT], BF16, tag="dj")
                    # dj = (recon + bc2) - p  (sign-flipped diff; squared)
                    nc.vector.scalar_tensor_tensor(
                        dj, psr, bc2[:, j:j + 1], pt[:, j, :],
                        op0=ALU.add, op1=ALU.subtract)
                    dsq = sm.tile([128, NT], BF16, tag="dsq")
                    nc.scalar.activation(dsq, dj, AF.Square)
                    nc.tensor.matmul(ms, ones, dsq,
                                     start=(j == 0), stop=(j == KD - 1))
                nc.vector.tensor_copy(z4[96:97, :], ms)

                # --- dissonance
                hd = big.tile([128, KD, NT], BF16, tag="hd")
                for j in range(KD):
                    ps = mm.tile([128, NT], F32, tag="mm")
                    for k in range(16):
                        x = pt[:, k, :] if k < KD else at[:, k - KD, :]
                        nc.tensor.matmul(ps, wdk(k)[:, j * 128:(j + 1) * 128],
                                         x, start=(k == 0), stop=(k == 15))
                    nc.scalar.activation(hd[:, j, :], ps, AF.Relu,
                                         bias=bd1[:, j:j + 1])
                zd = vec.tile([1, NT], F32, tag="vec")
                for j in range(KD):
                    nc.tensor.matmul(zd, wd2[:, j, :], hd[:, j, :],
                                     start=(j == 0), stop=(j == KD - 1))
                nc.scalar.activation(z4[0:1, :], zd, AF.Identity,
                                     bias=bh[0:1, 0:1])

                # --- move the 4 per-row scalars row-major: one [128,128]
                # PE transpose per subblock; quantities land on cols
                # {0,32,64,96} (zd, zu, zn, ms).
                zAll = sm.tile([128, 4, 4], F32, tag="zAll")
                for s in range(4):
                    ztr = trp.tile([128, 4, 32], F32, tag="ztr")
                    nc.tensor.transpose(
                        ztr, z4[:, s * 128:(s + 1) * 128], ident)
                    nc.vector.tensor_copy(zAll[:, s, :], ztr[:, :, 0])

                # --- per-tile finishing, row-major [128 rows, 4 subs]
                zdR = zAll[:, :, 0]
                zuR = zAll[:, :, 1]
                znR = zAll[:, :, 2]
                msR = zAll[:, :, 3]
                fin = sm.tile([128, 12, 4], F32, tag="fin")
                tmp1, tmp2, tmp3 = fin[:, 3, :], fin[:, 4, :], fin[:, 5, :]
                uncR, novR = fin[:, 6, :], fin[:, 7, :]
                tmp4, tmp5 = fin[:, 8, :], fin[:, 9, :]

                # softplus(z + b) = Ln(1 + Exp(z + b)); head biases were
                # already folded in by the z4 staging copies. One batched
                # Exp + Ln over all three heads [128, 4 subs, 3].
                sp3 = sm.tile([128, 4, 3], F32, tag="sp3")
                ex3 = sm.tile([128, 4, 3], F32, tag="ex3")
                nc.scalar.activation(ex3, zAll[:, :, 0:3], AF.Exp)
                nc.scalar.activation(sp3, ex3, AF.Ln, bias=1.0)
                spD, spU, spN = sp3[:, :, 0], sp3[:, :, 1], sp3[:, :, 2]

                # uncertainty = spU + 0.1 * (lnZ - zS/Z)
                nc.scalar.activation(tmp1, zZ, AF.Ln)      # lnZ
                nc.vector.reciprocal(tmp2, zZ)             # 1/Z
                nc.vector.tensor_mul(tmp3, zS, tmp2)       # zS/Z
                nc.vector.tensor_tensor(tmp1, tmp1, tmp3, ALU.subtract)
                nc.vector.scalar_tensor_tensor(
                    uncR, tmp1, 0.1, spU, op0=ALU.mult, op1=ALU.add)

                # novelty = 0.7*(1 - raw/||a||) + 0.3*spN
                nc.scalar.activation(tmp4, na2, AF.Ln)
                nc.scalar.activation(tmp5, tmp4, AF.Exp, scale=-0.5)  # 1/||a||
                nc.vector.tensor_mul(tmp4, raw, tmp5)      # cos
                nc.vector.tensor_scalar_mul(tmp5, spN, 0.3)
                nc.vector.scalar_tensor_tensor(
                    novR, tmp4, -0.7, tmp5, op0=ALU.mult, op1=ALU.add)
                nc.vector.tensor_scalar_add(novR, novR, 0.7)

                nc.vector.tensor_scalar_mul(tmp5, msR, 1.0 / D)     # comp

                # --- assemble this tile's [4, NT] output slice
                rs = slice(t * NT, (t + 1) * NT)
                for q, src in enumerate((spD, uncR, novR, tmp5)):
                    oT = trp.tile([4, 128], F32, tag="oT")
                    nc.tensor.transpose(oT, src, ident)
                    ob = sm.tile([4, 128], F32, tag="ob")
                    nc.scalar.copy(ob, oT)
                    nc.sync.dma_start(
                        out_d[q:q + 1, rs].rearrange(
                            "a (s r) -> (a s) r", s=4),
                        ob)

    _split_excess_waits(nc)
    return nc


def _prep_inputs(prediction, actual, pattern_memory,
                 W_d1, b_d1, W_d2, b_d2, W_u1, b_u1, W_u2, b_u2,
                 W_n1, b_n1, W_n2, b_n2, W_c1, b_c1, W_c2, b_c2):
    bf = ml_dtypes.bfloat16

    def t_bf(a):  # transposed contiguous bf16
        return np.ascontiguousarray(np.asarray(a, np.float32).T).astype(bf)

    mnorm = np.maximum(np.linalg.norm(
        np.asarray(pattern_memory, np.float32), axis=1), 1e-8)
    mhat = np.asarray(pattern_memory, np.float32) / mnorm[:, None]

    def fold_bias(b, chunks):
        return np.ascontiguousarray(
            np.asarray(b, np.float32).reshape(chunks, 128).T)

    bh = np.empty((128, 3), np.float32)
    bh[:, 0] = float(np.asarray(b_d2).reshape(-1)[0])
    bh[:, 1] = float(np.asarray(b_u2).reshape(-1)[0])
    bh[:, 2] = float(np.asarray(b_n2).reshape(-1)[0])

    shared = {
        "wd": t_bf(W_d1), "wu": t_bf(W_u1), "wn": t_bf(W_n1),
        "wc1": t_bf(W_c1), "wc2": t_bf(W_c2),
        "wd2": t_bf(W_d2), "wu2": t_bf(W_u2), "wn2": t_bf(W_n2),
        "mh": t_bf(mhat),
        "ones": np.ones((128, 1), bf),
        "ident": np.eye(128, dtype=np.float32),
        "bd1": fold_bias(b_d1, KD), "bu1": fold_bias(b_u1, 4),
        "bn1": fold_bias(b_n1, 4), "bc1": fold_bias(b_c1, 2),
        "bc2": fold_bias(b_c2, KD),
        "bh": bh,
    }
    p32 = np.asarray(prediction, np.float32)
    a32 = np.asarray(actual, np.float32)
    in_maps = []
    for c in range(NCORES):
        rows = slice(c * ROWS, (c + 1) * ROWS)
        m = dict(shared)
        m["pt"] = np.ascontiguousarray(p32[rows].T).astype(bf)
        m["at"] = np.ascontiguousarray(a32[rows].T).astype(bf)
        m["prm"] = np.ascontiguousarray(
            p32[rows].reshape(NSUB, 128, D).transpose(1, 0, 2)).astype(bf)
        m["arm"] = np.ascontiguousarray(
            a32[rows].reshape(NSUB, 128, D).transpose(1, 0, 2)).astype(bf)
        in_maps.append(m)
    return in_maps


_NC_CACHE = {}


def kernel(**inputs) -> np.ndarray:
    in_maps = _prep_inputs(**inputs)
    if 'nc' not in _NC_CACHE:
        _NC_CACHE['nc'] = build_kernel(reps=1)
    nc = _NC_CACHE['nc']
    res = run_bass_kernel_spmd(nc, in_maps, core_ids=list(range(NCORES)))
    out = np.empty((B, 4), np.float32)
    for c in range(NCORES):
        out[c * ROWS:(c + 1) * ROWS, :] = res.results[c]["out"].T
    return out


# revision 34
# speedup vs baseline: 1.0582x; 1.0145x over previous
"""Trainium2 Bass kernel for IntrinsicSignalSynthesizer.

Data-parallel over 8 NeuronCores: batch 16384 -> 8 x 2048 rows.

Design notes (v5):
- MLP matmuls are feature-major ([128 feat, kchunk, rows] tiles): every
  matmul contracts over the partition dim at the bf16 stream roofline.
- Per-row reductions (sum e^p, sum p*e^p, sum a^2) run OFF the PE: the
  inputs are DMA'd a second time in row-major layout and reduced along
  the free dim with DVE tensor_scalar/scalar_tensor_tensor accum_out.
- Per-row scalar finishing is row-major [128 rows, 4 subblocks] per
  row-tile; the PE-produced [1, 512] scalars (3 MLP heads + compression
  sum) are staged at quadrant partitions {0,32,64,96} of one tile and
  moved row-major with a single [128,128] PE transpose per subblock.
- DMA order: pattern memory + tile-0 feature-major inputs first, then
  small weights, then the 4 MiB dissonance weight in 4 chunks - the PE
  starts on sims/small MLPs ~4 us in while the big weight streams.
- Finishing is per-row-tile so the epilogue of the last tile is short.
"""
import sys
sys.path.insert(0, '/opt/trn_rl_repo')

import numpy as np
import ml_dtypes

import concourse.bass as bass
import concourse.mybir as mybir
import concourse.tile as tile
from concourse.bass_utils import run_bass_kernel_spmd

BF16 = mybir.dt.bfloat16
F32 = mybir.dt.float32
AF = mybir.ActivationFunctionType
ALU = mybir.AluOpType
AX = mybir.AxisListType

B, D = 16384, 1024
MEM = 100
NCORES = 8
ROWS = B // NCORES            # 2048 rows per core
NT = 512                      # rows per row-tile
NTILES = ROWS // NT           # 4
NSUB = ROWS // 128            # 16 row-subblocks per core
KD = D // 128                 # 8 feature chunks

MAX_WAITS = 1


def _split_excess_waits(nc):
    # walrus CTRL encoding caps sync waits per instruction; the TileContext
    # tail drain can exceed that. Move excess waits onto preceding NoOps.
    for fn in nc.m.functions:
        for bb in fn.blocks:
            if not isinstance(bb, mybir.BasicBlock):
                continue
            insts = bb.instructions
            i = 0
            while i < len(insts):
                ins = insts[i]
                si = getattr(ins, 'sync_info', None)
                waits = list(si.on_wait) if si is not None and si.on_wait else []
                if len(waits) > MAX_WAITS:
                    chunks = [waits[j:j + MAX_WAITS]
                              for j in range(0, len(waits), MAX_WAITS)]
                    si.on_wait = chunks[-1]
                    new_ops = [
                        mybir.InstNoOp(
                            name=f"{ins.name}-waitsplit-{k}",
                            engine=ins.engine,
                            sync_info=mybir.SyncInfo(on_wait=ch, on_update=[]),
                            bass_nofuse=True,
                        )
                        for k, ch in enumerate(chunks[:-1])
                    ]
                    insts[i:i] = new_ops
                    i += len(new_ops)
                i += 1


def build_kernel(reps: int = 1):
    assert reps == 1, "tile-0 input prefetch assumes a single pass"
    nc = bass.Bass()

    pt_d = nc.dram_tensor("pt", [D, ROWS], BF16, kind="ExternalInput")
    at_d = nc.dram_tensor("at", [D, ROWS], BF16, kind="ExternalInput")
    prm_d = nc.dram_tensor("prm", [128, NSUB, D], BF16, kind="ExternalInput")
    arm_d = nc.dram_tensor("arm", [128, NSUB, D], BF16, kind="ExternalInput")
    wd_d = nc.dram_tensor("wd", [2 * D, D], BF16, kind="ExternalInput")
    wu_d = nc.dram_tensor("wu", [D, D // 2], BF16, kind="ExternalInput")
    wn_d = nc.dram_tensor("wn", [D, D // 2], BF16, kind="ExternalInput")
    wc1_d = nc.dram_tensor("wc1", [D, D // 4], BF16, kind="ExternalInput")
    wc2_d = nc.dram_tensor("wc2", [D // 4, D], BF16, kind="ExternalInput")
    wd2_d = nc.dram_tensor("wd2", [D, 1], BF16, kind="ExternalInput")
    wu2_d = nc.dram_tensor("wu2", [D // 2, 1], BF16, kind="ExternalInput")
    wn2_d = nc.dram_tensor("wn2", [D // 2, 1], BF16, kind="ExternalInput")
    mh_d = nc.dram_tensor("mh", [D, MEM], BF16, kind="ExternalInput")
    ones_d = nc.dram_tensor("ones", [128, 1], BF16, kind="ExternalInput")
    ident_d = nc.dram_tensor("ident", [128, 128], F32, kind="ExternalInput")
    bd1_d = nc.dram_tensor("bd1", [128, KD], F32, kind="ExternalInput")
    bu1_d = nc.dram_tensor("bu1", [128, 4], F32, kind="ExternalInput")
    bn1_d = nc.dram_tensor("bn1", [128, 4], F32, kind="ExternalInput")
    bc1_d = nc.dram_tensor("bc1", [128, 2], F32, kind="ExternalInput")
    bc2_d = nc.dram_tensor("bc2", [128, KD], F32, kind="ExternalInput")
    bh_d = nc.dram_tensor("bh", [128, 3], F32, kind="ExternalInput")  # d2,u2,n2
    out_d = nc.dram_tensor("out", [4, ROWS], F32, kind="ExternalOutput")

    with tile.TileContext(nc) as tc:
        import contextlib
        ctx = contextlib.ExitStack()
        with ctx:
            W = ctx.enter_context(tc.tile_pool(name="weights", bufs=1))
            io = ctx.enter_context(tc.tile_pool(name="io", bufs=3))
            io2 = ctx.enter_context(tc.tile_pool(name="io2", bufs=2))
            big = ctx.enter_context(tc.tile_pool(name="big", bufs=2))
            sm = ctx.enter_context(tc.tile_pool(name="sm", bufs=2))
            mm = ctx.enter_context(tc.tile_pool(name="mmp", bufs=3, space="PSUM"))
            vec = ctx.enter_context(tc.tile_pool(name="vecp", bufs=2, space="PSUM"))
            simp = ctx.enter_context(tc.tile_pool(name="simp", bufs=1, space="PSUM"))
            trp = ctx.enter_context(tc.tile_pool(name="trp", bufs=1, space="PSUM"))

            # --- DMA emission order == start order. First what the PE needs
            # first: pattern memory, tile-0 FM inputs, small weights; the big
            # dissonance weight last, streaming under early compute.
            mh = W.tile([128, KD, MEM], BF16)
            nc.sync.dma_start(mh, mh_d.rearrange("(k p) m -> p k m", p=128))

            def load_at(t):
                rs = slice(t * NT, (t + 1) * NT)
                at = io.tile([128, KD, NT], BF16, tag="at")
                nc.sync.dma_start(
                    at, at_d[:, rs].rearrange("(k p) r -> p k r", p=128))
                return at

            def load_pt(t):
                rs = slice(t * NT, (t + 1) * NT)
                pt = io.tile([128, KD, NT], BF16, tag="pt")
                nc.sync.dma_start(
                    pt, pt_d[:, rs].rearrange("(k p) r -> p k r", p=128))
                return pt

            def load_rm(t):
                prm = io2.tile([128, 4, D], BF16, tag="prm")
                nc.sync.dma_start(prm, prm_d[:, 4 * t:4 * t + 4, :])
                arm = io2.tile([128, 4, D], BF16, tag="arm")
                nc.sync.dma_start(arm, arm_d[:, 4 * t:4 * t + 4, :])
                return prm, arm

            at0 = load_at(0)

            wn = W.tile([128, KD, D // 2], BF16)
            nc.sync.dma_start(wn, wn_d.rearrange("(k p) m -> p k m", p=128))
            wu = W.tile([128, KD, D // 2], BF16)
            nc.sync.dma_start(wu, wu_d.rearrange("(k p) m -> p k m", p=128))
            pt0 = load_pt(0)
            wc1 = W.tile([128, KD, D // 4], BF16)
            nc.sync.dma_start(wc1, wc1_d.rearrange("(k p) m -> p k m", p=128))
            wc2 = W.tile([128, 2, D], BF16)
            nc.sync.dma_start(wc2, wc2_d.rearrange("(k p) m -> p k m", p=128))
            wd2 = W.tile([128, KD, 1], BF16)
            nc.sync.dma_start(wd2, wd2_d.rearrange("(k p) m -> p k m", p=128))
            wu2 = W.tile([128, 4, 1], BF16)
            nc.sync.dma_start(wu2, wu2_d.rearrange("(k p) m -> p k m", p=128))
            wn2 = W.tile([128, 4, 1], BF16)
            nc.sync.dma_start(wn2, wn2_d.rearrange("(k p) m -> p k m", p=128))
            ones = W.tile([128, 1], BF16)
            nc.sync.dma_start(ones, ones_d[:])
            ident = W.tile([128, 128], F32)
            nc.sync.dma_start(ident, ident_d[:])
            bd1 = W.tile([128, KD], F32)
            nc.sync.dma_start(bd1, bd1_d[:])
            bu1 = W.tile([128, 4], F32)
            nc.sync.dma_start(bu1, bu1_d[:])
            bn1 = W.tile([128, 4], F32)
            nc.sync.dma_start(bn1, bn1_d[:])
            bc1 = W.tile([128, 2], F32)
            nc.sync.dma_start(bc1, bc1_d[:])
            bc2 = W.tile([128, KD], F32)
            nc.sync.dma_start(bc2, bc2_d[:])
            bh = W.tile([128, 3], F32)
            nc.sync.dma_start(bh, bh_d[:])
            rm0 = load_rm(0)
            # dissonance weight last (4 chunk-group DMAs), streaming in under
            # the sims/small-MLP work of tile 0
            wd = []
            for g in range(4):
                wg = W.tile([128, 4, D], BF16, tag=f"wdc{g}")
                nc.sync.dma_start(
                    wg, wd_d[g * 512:(g + 1) * 512, :].rearrange(
                        "(k p) m -> p k m", p=128))
                wd.append(wg)

            def wdk(k):
                return wd[k // 4][:, k % 4, :]

            for t in range(NTILES):
                at = at0 if t == 0 else load_at(t)
                pt = pt0 if t == 0 else load_pt(t)
                prm, arm = rm0 if t == 0 else load_rm(t)
                raw = sm.tile([128, 4], F32, tag="raw")
                zZ = sm.tile([128, 4], F32, tag="zZ")
                zS = sm.tile([128, 4], F32, tag="zS")
                na2 = sm.tile([128, 4], F32, tag="na2")

                # --- sims (needs only mh + at): row-major max cos sim
                for s in range(4):
                    pss = simp.tile([128, MEM], F32, tag="simp")
                    for k in range(KD):
                        nc.tensor.matmul(
                            pss, at[:, k, s * 128:(s + 1) * 128],
                            mh[:, k, :],
                            start=(k == 0), stop=(k == KD - 1))
                    nc.vector.reduce_max(raw[:, s:s + 1], pss, axis=AX.X)

                # per-row scalars staged at quadrant-aligned partitions
                # {0,32,64,96} (engine writes must be 32-aligned)
                z4 = sm.tile([128, NT], F32, tag="z4")

                # --- novelty MLP
                hn = big.tile([128, 4, NT], BF16, tag="hn")
                for j in range(4):
                    ps = mm.tile([128, NT], F32, tag="mm")
                    for k in range(KD):
                        nc.tensor.matmul(ps, wn[:, k, j * 128:(j + 1) * 128],
                                         at[:, k, :],
                                         start=(k == 0), stop=(k == KD - 1))
                    nc.scalar.activation(hn[:, j, :], ps, AF.Relu,
                                         bias=bn1[:, j:j + 1])
                zn = vec.tile([1, NT], F32, tag="vec")
                for j in range(4):
                    nc.tensor.matmul(zn, wn2[:, j, :], hn[:, j, :],
                                     start=(j == 0), stop=(j == 3))
                nc.scalar.activation(z4[64:65, :], zn, AF.Identity,
                                     bias=bh[0:1, 2:3])

                # --- uncertainty MLP
                hu = big.tile([128, 4, NT], BF16, tag="hu")
                for j in range(4):
                    ps = mm.tile([128, NT], F32, tag="mm")
                    for k in range(KD):
                        nc.tensor.matmul(ps, wu[:, k, j * 128:(j + 1) * 128],
                                         pt[:, k, :],
                                         start=(k == 0), stop=(k == KD - 1))
                    nc.scalar.activation(hu[:, j, :], ps, AF.Relu,
                                         bias=bu1[:, j:j + 1])
                zu = vec.tile([1, NT], F32, tag="vec")
                for j in range(4):
                    nc.tensor.matmul(zu, wu2[:, j, :], hu[:, j, :],
                                     start=(j == 0), stop=(j == 3))
                nc.scalar.activation(z4[32:33, :], zu, AF.Identity,
                                     bias=bh[0:1, 1:2])

                # --- entropy sums + ||a||^2: row-major free-dim reduces
                # fused into DVE elementwise ops via accum_out
                for s in range(4):
                    e = sm.tile([128, D], BF16, tag="e")
                    nc.scalar.activation(e, prm[:, s, :], AF.Exp)
                    e2 = sm.tile([128, D], BF16, tag="e2")
                    nc.vector.tensor_scalar(
                        e2, e, 1.0, 0.0, op0=ALU.mult, op1=ALU.add,
                        accum_out=zZ[:, s:s + 1])
                    nc.vector.scalar_tensor_tensor(
                        e2, e, 1.0, prm[:, s, :],
                        op0=ALU.mult, op1=ALU.mult,
                        accum_out=zS[:, s:s + 1])
                    nc.vector.scalar_tensor_tensor(
                        e2, arm[:, s, :], 1.0, arm[:, s, :],
                        op0=ALU.mult, op1=ALU.mult,
                        accum_out=na2[:, s:s + 1])

                # --- compression
                hc = big.tile([128, 2, NT], BF16, tag="hc")
                for j in range(2):
                    ps = mm.tile([128, NT], F32, tag="mm")
                    for k in range(KD):
                        nc.tensor.matmul(ps, wc1[:, k, j * 128:(j + 1) * 128],
                                         pt[:, k, :],
                                         start=(k == 0), stop=(k == KD - 1))
                    nc.scalar.activation(hc[:, j, :], ps, AF.Relu,
                                         bias=bc1[:, j:j + 1])
                ms = vec.tile([1, NT], F32, tag="vec")
                for j in range(KD):
                    psr = mm.tile([128, NT], F32, tag="mm")
                    for k in range(2):
                        nc.tensor.matmul(psr, wc2[:, k, j * 128:(j + 1) * 128],
                                         hc[:, k, :],
                                         start=(k == 0), stop=(k == 1))
                    dj = sm.tile([128, NT], BF16, tag="dj")
                    # dj = (recon + bc2) - p  (sign-flipped diff; squared)
                    nc.vector.scalar_tensor_tensor(
                        dj, psr, bc2[:, j:j + 1], pt[:, j, :],
                        op0=ALU.add, op1=ALU.subtract)
                    dsq = sm.tile([128, NT], BF16, tag="dsq")
                    nc.scalar.activation(dsq, dj, AF.Square)
                    nc.tensor.matmul(ms, ones, dsq,
                                     start=(j == 0), stop=(j == KD - 1))
                nc.vector.tensor_copy(z4[96:97, :], ms)

                # --- dissonance
                hd = big.tile([128, KD, NT], BF16, tag="hd")
                for j in range(KD):
                    ps = mm.tile([128, NT], F32, tag="mm")
                    for k in range(16):
                        x = pt[:, k, :] if k < KD else at[:, k - KD, :]
                        nc.tensor.matmul(ps, wdk(k)[:, j * 128:(j + 1) * 128],
                                         x, start=(k == 0), stop=(k == 15))
                    nc.scalar.activation(hd[:, j, :], ps, AF.Relu,
                                         bias=bd1[:, j:j + 1])
                zd = vec.tile([1, NT], F32, tag="vec")
                for j in range(KD):
                    nc.tensor.matmul(zd, wd2[:, j, :], hd[:, j, :],
                                     start=(j == 0), stop=(j == KD - 1))
                nc.scalar.activation(z4[0:1, :], zd, AF.Identity,
                                     bias=bh[0:1, 0:1])

                # --- move the 4 per-row scalars row-major: one [128,128]
                # PE transpose per subblock; quantities land on cols
                # {0,32,64,96} (zd, zu, zn, ms).
                zAll = sm.tile([128, 4, 4], F32, tag="zAll")
                for s in range(4):
                    ztr = trp.tile([128, 4, 32], F32, tag="ztr")
                    nc.tensor.transpose(
                        ztr, z4[:, s * 128:(s + 1) * 128], ident)
                    nc.vector.tensor_copy(zAll[:, s, :], ztr[:, :, 0])

                # --- per-tile finishing, row-major [128 rows, 4 subs]
                zdR = zAll[:, :, 0]
                zuR = zAll[:, :, 1]
                znR = zAll[:, :, 2]
                msR = zAll[:, :, 3]
                fin = sm.tile([128, 12, 4], F32, tag="fin")
                tmp1, tmp2, tmp3 = fin[:, 3, :], fin[:, 4, :], fin[:, 5, :]
                tmp4, tmp5 = fin[:, 8, :], fin[:, 9, :]
                # final quantities laid out [128, quantity, sub] so one
                # transpose + one DMA emits the whole [4, NT] output slice
                finQ = sm.tile([128, 4, 4], F32, tag="finQ")
                uncR, novR = finQ[:, 1, :], finQ[:, 2, :]

                # softplus(z + b) = Ln(1 + Exp(z + b)); head biases were
                # already folded in by the z4 staging copies. One batched
                # Exp + Ln over all three heads [128, 4 subs, 3].
                sp3 = sm.tile([128, 4, 3], F32, tag="sp3")
                ex3 = sm.tile([128, 4, 3], F32, tag="ex3")
                nc.scalar.activation(ex3, zAll[:, :, 0:3], AF.Exp)
                nc.scalar.activation(sp3, ex3, AF.Ln, bias=1.0)
                spD, spU, spN = sp3[:, :, 0], sp3[:, :, 1], sp3[:, :, 2]
                nc.vector.tensor_copy(finQ[:, 0, :], spD)

                # uncertainty = spU + 0.1 * (lnZ - zS/Z)
                nc.scalar.activation(tmp1, zZ, AF.Ln)      # lnZ
                nc.vector.reciprocal(tmp2, zZ)             # 1/Z
                nc.vector.tensor_mul(tmp3, zS, tmp2)       # zS/Z
                nc.vector.tensor_tensor(tmp1, tmp1, tmp3, ALU.subtract)
                nc.vector.scalar_tensor_tensor(
                    uncR, tmp1, 0.1, spU, op0=ALU.mult, op1=ALU.add)

                # novelty = 0.7*(1 - raw/||a||) + 0.3*spN
                nc.scalar.activation(tmp4, na2, AF.Ln)
                nc.scalar.activation(tmp5, tmp4, AF.Exp, scale=-0.5)  # 1/||a||
                nc.vector.tensor_mul(tmp4, raw, tmp5)      # cos
                nc.vector.tensor_scalar_mul(tmp5, spN, 0.3)
                nc.vector.scalar_tensor_tensor(
                    novR, tmp4, -0.7, tmp5, op0=ALU.mult, op1=ALU.add)
                nc.vector.tensor_scalar_add(novR, novR, 0.7)

                nc.vector.tensor_scalar_mul(finQ[:, 3, :], msR, 1.0 / D)

                # --- assemble this tile's [4, NT] output slice with one
                # [128,16] transpose and one 16-line DMA
                rs = slice(t * NT, (t + 1) * NT)
                oT = trp.tile([16, 128], F32, tag="oT")
                nc.tensor.transpose(
                    oT, finQ.rearrange("p q s -> p (q s)"), ident)
                ob = sm.tile([16, 128], F32, tag="ob")
                nc.scalar.copy(ob, oT)
                for q in range(4):
                    nc.sync.dma_start(
                        out_d[q:q + 1, rs].rearrange("a (s r) -> (a s) r", s=4),
                        ob[4 * q:4 * q + 4, :])

    _split_excess_waits(nc)
    return nc


def _prep_inputs(prediction, actual, pattern_memory,
                 W_d1, b_d1, W_d2, b_d2, W_u1, b_u1, W_u2, b_u2,
                 W_n1, b_n1, W_n2, b_n2, W_c1, b_c1, W_c2, b_c2):
    bf = ml_dtypes.bfloat16

    def t_bf(a):  # transposed contiguous bf16
        return np.ascontiguousarray(np.asarray(a, np.float32).T).astype(bf)

    mnorm = np.maximum(np.linalg.norm(
        np.asarray(pattern_memory, np.float32), axis=1), 1e-8)
    mhat = np.asarray(pattern_memory, np.float32) / mnorm[:, None]

    def fold_bias(b, chunks):
        return np.ascontiguousarray(
            np.asarray(b, np.float32).reshape(chunks, 128).T)

    bh = np.empty((128, 3), np.float32)
    bh[:, 0] = float(np.asarray(b_d2).reshape(-1)[0])
    bh[:, 1] = float(np.asarray(b_u2).reshape(-1)[0])
    bh[:, 2] = float(np.asarray(b_n2).reshape(-1)[0])

    shared = {
        "wd": t_bf(W_d1), "wu": t_bf(W_u1), "wn": t_bf(W_n1),
        "wc1": t_bf(W_c1), "wc2": t_bf(W_c2),
        "wd2": t_bf(W_d2), "wu2": t_bf(W_u2), "wn2": t_bf(W_n2),
        "mh": t_bf(mhat),
        "ones": np.ones((128, 1), bf),
        "ident": np.eye(128, dtype=np.float32),
        "bd1": fold_bias(b_d1, KD), "bu1": fold_bias(b_u1, 4),
        "bn1": fold_bias(b_n1, 4), "bc1": fold_bias(b_c1, 2),
        "bc2": fold_bias(b_c2, KD),
        "bh": bh,
    }
    p32 = np.asarray(prediction, np.float32)
    a32 = np.asarray(actual, np.float32)
    in_maps = []
    for c in range(NCORES):
        rows = slice(c * ROWS, (c + 1) * ROWS)
        m = dict(shared)
        m["pt"] = np.ascontiguousarray(p32[rows].T).astype(bf)
        m["at"] = np.ascontiguousarray(a32[rows].T).astype(bf)
        m["prm"] = np.ascontiguousarray(
            p32[rows].reshape(NSUB, 128, D).transpose(1, 0, 2)).astype(bf)
        m["arm"] = np.ascontiguousarray(
            a32[rows].reshape(NSUB, 128, D).transpose(1, 0, 2)).astype(bf)
        in_maps.append(m)
    return in_maps


_NC_CACHE = {}


def kernel(**inputs) -> np.ndarray:
    in_maps = _prep_inputs(**inputs)
    if 'nc' not in _NC_CACHE:
        _NC_CACHE['nc'] = build_kernel(reps=1)
    nc = _NC_CACHE['nc']
    res = run_bass_kernel_spmd(nc, in_maps, core_ids=list(range(NCORES)))
    out = np.empty((B, 4), np.float32)
    for c in range(NCORES):
        out[c * ROWS:(c + 1) * ROWS, :] = res.results[c]["out"].T
    return out
